# revision 29
# baseline (speedup 1.0000x reference)
"""Causal grouped-query paged attention (prefill) on 8 Trainium2 NeuronCores.

Problem (hardcoded): T=4096 tokens (B=2 seqs x SEQ=2048), 32 q heads,
8 kv heads (GQA group g=4), head_dim=128, paged fp32 KV cache
(512 blocks x 16 tokens).

Sharding: tensor-parallel over KV heads -- core h gets kv head h, its 4
query heads, and both sequences => 8 causal attention slices of
[2048 q x 2048 k x 128 d] per core.

Device kernel v2 (per core):
  - S^T orientation: scores^T[k, q] with K chunks stationary, Q^T moving;
    PV consumes P^T directly (no transposes).
  - NO denominator matmul and NO on-device epilogue: the softmax
    denominator partials P_sum[p, q] = sum_chunks P^T are accumulated on
    the (otherwise idle) Vector engine in fp16 and DMA'd out; the host
    finishes the cross-partition sum and normalizes O^T / ds. Device
    output is unnormalized O^T fp16 + P_sum fp16.
  - exp batched: S chunks for up to 3 full 512-wide chunks (or a whole
    1280-wide diagonal pack) land in one 3-bank PSUM tile and are
    activated by a single ScalarE instruction, amortizing the ~150ns
    per-instruction activation overhead.
  - exact causal diagonal: chunk widths 512/384/128/256 (j order
    0,1,3,2 packs tightly with no PSUM bank crossings), triangles
    masked by one 128x128 VectorE multiply each.
  - software pipelining: S matmuls of group g are emitted before the
    PV matmuls of group g-1 so the PE never sits behind the activation.
"""

import math

import numpy as np

import concourse.bass as bass
import concourse.tile as tile
from concourse import bacc, mybir
from concourse.bass_utils import run_bass_kernel_spmd
from concourse.masks import make_identity

# problem constants
B = 2
SEQ = 2048
T = B * SEQ
N_QO_HEADS = 32
N_KV_HEADS = 8
G = N_QO_HEADS // N_KV_HEADS  # 4
D = 128
PAGE = 16
NUM_BLOCKS = 512
N_CORES = 8

QTILE = 512   # q tile (PSUM ot bank width)
KCH = 128     # k chunk (PE contraction tile)
GW = 1536     # exp group width (3 PSUM banks)
F32 = mybir.dt.float32
FP16 = mybir.dt.float16
SM_SCALE = 1.0 / math.sqrt(D)


def _build_groups(n_slices, nq, slice_to_seq):
    """Flat list of exp groups. Each group: dict(s, b, qc, chunks, span,
    qtile_first, qtile_last). chunk = (kc, q0, w, off, isdiag) where q0 is
    the q offset inside the qtile and off the column offset in the PSUM
    group tile. Chunks never cross a 512-col PSUM bank boundary."""
    groups = []
    for s in range(n_slices):
        b = slice_to_seq[s]
        # descending qtile order: the big qc needs only this slice's qt
        # low..high cols already resident; later slices' qt DMAs get more
        # time to land, and each slice drains with its smallest qtile
        for qc in reversed(range(nq)):
            qgroups = []
            fulls = list(range(4 * qc))
            # batch sizes chosen to keep exp group spans even (avoid a
            # lone 512-span group causing pipeline hiccups)
            batching = {0: [], 4: [2, 2], 8: [2, 3, 3], 12: [3, 3, 3, 3]}[len(fulls)]
            i0 = 0
            for nb in batching:
                batch = fulls[i0 : i0 + nb]
                i0 += nb
                chunks = [
                    (kc, 0, QTILE, i * QTILE, False) for i, kc in enumerate(batch)
                ]
                qgroups.append(
                    dict(s=s, b=b, qc=qc, chunks=chunks, span=QTILE * len(batch))
                )
            base = 4 * qc
            diag = [
                (base + 0, 0, 512, 0, True),
                (base + 1, 128, 384, 512, True),
                (base + 3, 384, 128, 896, True),
                (base + 2, 256, 256, 1024, True),
            ]
            qgroups.append(dict(s=s, b=b, qc=qc, chunks=diag, span=1280))
            qgroups[0]["qtile_first"] = True
            qgroups[-1]["qtile_last"] = True
            groups.extend(qgroups)
    return groups


def emit(nc, n_slices, n_seqs, seq, slice_to_seq):
    """Inputs (DRAM):
      qt   [n_slices, 128, seq]  Q^T per slice (fp16)
      kt   [n_seqs,   128, seq]  K^T per sequence (fp16)
      v    [n_seqs,   128, seq]  V per sequence, chunk-packed (fp16)
      mask [128, 128]            upper-tri ones (fp16)
    Outputs:
      o  [n_slices, 128, seq]  unnormalized O^T (fp16)
      ps [n_slices, 128, seq]  softmax denominator partials (fp16)
    """
    nq = seq // QTILE

    qt = nc.dram_tensor("qt", [n_slices, D, seq], FP16, kind="ExternalInput").ap()
    kt = nc.dram_tensor("kt", [n_seqs, D, seq], FP16, kind="ExternalInput").ap()
    v = nc.dram_tensor("v", [n_seqs, seq, D], FP16, kind="ExternalInput").ap()
    mask = nc.dram_tensor("mask", [D, D], FP16, kind="ExternalInput").ap()
    o = nc.dram_tensor("o", [n_slices, D, seq], FP16, kind="ExternalOutput").ap()
    ps = nc.dram_tensor("ps", [n_slices, D, seq], FP16, kind="ExternalOutput").ap()

    groups = _build_groups(n_slices, nq, slice_to_seq)

    with tile.TileContext(nc) as tc:
        with (
            tc.tile_pool(name="const", bufs=1) as const_pool,
            tc.tile_pool(name="kv", bufs=1) as kv_pool,
            tc.tile_pool(name="q", bufs=1) as q_pool,
            tc.tile_pool(name="pt", bufs=4) as pt_pool,
            tc.tile_pool(name="psv", bufs=2) as psv_pool,
            tc.tile_pool(name="osb", bufs=2) as osb_pool,
            tc.tile_pool(name="st", bufs=2, space="PSUM") as st_pool,
            tc.tile_pool(name="ot", bufs=2, space="PSUM") as ot_pool,
        ):
            # warm the Exp activation table at t=0 so the first real exp
            # doesn't pay the ~1.3us ACT_TABLE_LOAD on the critical path
            warm = const_pool.tile([1, 8], F32, name="warm")
            nc.vector.memset(warm[:], 0)
            nc.scalar.activation(
                warm[:], warm[:], mybir.ActivationFunctionType.Exp
            )
            # mconst: -30000 strictly below the causal diagonal ([k,q]: q<k);
            # added into score PSUM via a tiny identity matmul so exp yields
            # exact zeros -- no post-exp masking dependency.
            mconst_sb = const_pool.tile([D, D], FP16, name="mconst_sb")
            nc.sync.dma_start(mconst_sb[:], mask[:])
            identity_h = const_pool.tile([D, D], FP16, name="identity_h")
            make_identity(nc, identity_h[:])
            kt_sb = []
            v_sb = []
            for b in range(n_seqs):
                kt_sb.append(kv_pool.tile([D, seq], FP16, tag=f"kt{b}", name=f"ktsb{b}"))
                v_sb.append(kv_pool.tile([D, seq], FP16, tag=f"v{b}", name=f"vsb{b}"))
            qt_sb = [
                q_pool.tile([D, seq], FP16, tag=f"qt{s}", name=f"qtsb{s}")
                for s in range(n_slices)
            ]
            b0 = slice_to_seq[0]
            loaded = set()
            def load_v(b, c0=0, c1=None):
                # v chunks packed along free dim: chunk c at cols [c*128, +128)
                c1 = seq // D if c1 is None else c1
                nc.sync.dma_start(
                    v_sb[b][:].rearrange("p (c d) -> p c d", d=D)[:, c0:c1, :],
                    v[b].rearrange("(c p) d -> p c d", p=D)[:, c0:c1, :],
                )

            def load_seq(b):
                if b in loaded:
                    return
                loaded.add(b)
                nc.sync.dma_start(kt_sb[b][:], kt[b])
                load_v(b)

            # priority prefix for the first (descending-order) qtile qc=3:
            # its full groups stream kt cols upward while its q columns sit
            # in the top quarter of qt[0]; disjoint pieces so the first
            # matmuls start early, remainder loads follow.
            # DMAs dispatched in compute need-order (SP dispatch is serial,
            # ~1us each; the first consumer of each piece sets its deadline)
            half = seq // 2
            nch = seq // D
            # first exp group (qc3, kc 0-2) needs only kt cols 0:384 and the
            # top qt quarter: tiny first pieces gate the very first matmuls
            nc.sync.dma_start(kt_sb[b0][:, 0:QTILE], kt[b0][:, 0:QTILE])
            nc.sync.dma_start(
                qt_sb[0][:, seq - QTILE : seq], qt[0][:, seq - QTILE : seq]
            )
            load_v(b0, c0=nch - 4)  # first PVs hit the qc3 diagonal chunks
            nc.sync.dma_start(kt_sb[b0][:, QTILE:half], kt[b0][:, QTILE:half])
            nc.sync.dma_start(kt_sb[b0][:, half:seq], kt[b0][:, half:seq])
            nc.sync.dma_start(qt_sb[0][:, 0 : seq - QTILE], qt[0][:, 0 : seq - QTILE])
            load_v(b0, c0=0, c1=nch - 4)
            loaded.add(b0)
            # slices 1-3 (still seq 0): upper halves needed first (qc desc)
            for s in range(1, 4):
                nc.sync.dma_start(qt_sb[s][:, half:seq], qt[s][:, half:seq])
                nc.sync.dma_start(qt_sb[s][:, 0:half], qt[s][:, 0:half])
            # seq 1 K/V: first needed by slice 4, far later
            for b in range(n_seqs):
                load_seq(b)
            for s in range(4, n_slices):
                nc.sync.dma_start(qt_sb[s][:, half:seq], qt[s][:, half:seq])
            for s in range(4, n_slices):
                nc.sync.dma_start(qt_sb[s][:, 0:half], qt[s][:, 0:half])

            state = {"ot": None, "psum": None}

            def produce(g):
                """S matmuls -> one exp -> triangle masks. Returns ptile."""
                s, b = g["s"], g["b"]
                qbase = g["qc"] * QTILE
                st_t = st_pool.tile([D, GW], F32, tag="st", name="st_t")
                ptile = pt_pool.tile([D, GW], FP16, tag="pt", name="ptile")
                for kc, q0, w, off, isdiag in g["chunks"]:
                    nc.tensor.matmul(
                        st_t[:, off : off + w],
                        lhsT=kt_sb[b][:, kc * KCH : (kc + 1) * KCH],
                        rhs=qt_sb[s][:, qbase + q0 : qbase + q0 + w],
                        start=True,
                        stop=not isdiag,
                    )
                    if isdiag:
                        # accumulate -30000 into the triangle (first 128 cols)
                        nc.tensor.matmul(
                            st_t[:, off : off + D],
                            lhsT=identity_h[:],
                            rhs=mconst_sb[:],
                            start=False,
                            stop=True,
                        )
                span = g["span"]
                nc.scalar.activation(
                    ptile[:, :span],
                    st_t[:, :span],
                    mybir.ActivationFunctionType.Exp,
                    scale=SM_SCALE,
                )
                return ptile

            def consume(g, ptile):
                """PV matmuls + P_sum accumulation; flush at qtile end."""
                s, b, qc = g["s"], g["b"], g["qc"]
                if g.get("qtile_first"):
                    state["ot"] = ot_pool.tile([D, QTILE], F32, tag="ot", name="ot_t")
                    state["psum"] = psv_pool.tile([D, QTILE], FP16, tag="psum", name="ps_t")
                ot_t, ps_t = state["ot"], state["psum"]
                first = g.get("qtile_first", False)
                last_g = g.get("qtile_last", False)
                nchunks = len(g["chunks"])
                for i, (kc, q0, w, off, isdiag) in enumerate(g["chunks"]):
                    nc.tensor.matmul(
                        ot_t[:, q0 : q0 + w],
                        lhsT=v_sb[b][:, kc * KCH : (kc + 1) * KCH],
                        rhs=ptile[:, off : off + w],
                        start=(first and i == 0),
                        stop=(last_g and i == nchunks - 1),
                    )
                    if first and i == 0:
                        nc.vector.tensor_copy(
                            ps_t[:, q0 : q0 + w], ptile[:, off : off + w]
                        )
                    else:
                        nc.vector.tensor_add(
                            ps_t[:, q0 : q0 + w],
                            ps_t[:, q0 : q0 + w],
                            ptile[:, off : off + w],
                        )
                if last_g:
                    osb = osb_pool.tile([D, QTILE], FP16, tag="osb", name="osb")
                    nc.vector.tensor_copy(osb[:], ot_t[:])
                    q0a = qc * QTILE
                    nc.sync.dma_start(o[s, :, q0a : q0a + QTILE], osb[:])
                    nc.sync.dma_start(ps[s, :, q0a : q0a + QTILE], ps_t[:])

            pending = None
            for g in groups:
                ptile = produce(g)
                if pending is not None:
                    consume(*pending)
                pending = (g, ptile)
            consume(*pending)
    return nc


_CACHE = {}


def _build_full():
    key = "full"
    if key not in _CACHE:
        nc = bacc.Bacc(
            "TRN2",
            target_bir_lowering=False,
            debug=False,
            enable_asserts=False,
            num_devices=N_CORES,
        )
        emit(nc, n_slices=B * G, n_seqs=B, seq=SEQ,
             slice_to_seq=[b for b in range(B) for _ in range(G)])
        nc.compile()
        _CACHE[key] = nc
    return _CACHE[key]


def shard_inputs(query, key, value, key_cache, value_cache, block_tables,
                 new_cache_slots):
    """Host-side scatter/gather + head sharding. Returns per-core input maps."""
    kc = key_cache.reshape(NUM_BLOCKS * PAGE, N_KV_HEADS, D).copy()
    vc = value_cache.reshape(NUM_BLOCKS * PAGE, N_KV_HEADS, D).copy()
    kc[new_cache_slots] = key.reshape(T, N_KV_HEADS, D)
    vc[new_cache_slots] = value.reshape(T, N_KV_HEADS, D)
    idx = (
        block_tables[:, :, None].astype(np.int64) * PAGE
        + np.arange(PAGE, dtype=np.int64)[None, None, :]
    ).reshape(B, SEQ)
    k_all = kc[idx]  # [B, SEQ, Hkv, D]
    v_all = vc[idx]
    q_all = query.reshape(B, SEQ, N_KV_HEADS, G, D)
    # -30000 strictly below the causal diagonal ([k, q]: q < k), 0 elsewhere;
    # added to raw scores pre-exp so masked positions exp to exactly 0
    mask = (-30000.0 * np.tril(np.ones((D, D)), -1)).astype(np.float16)

    bf = np.float16
    in_maps = []
    for h in range(N_CORES):
        qt = np.ascontiguousarray(
            q_all[:, :, h, :, :].transpose(0, 2, 3, 1).reshape(B * G, D, SEQ)
        ).astype(bf)
        kt = np.ascontiguousarray(k_all[:, :, h, :].transpose(0, 2, 1)).astype(bf)
        vv = np.ascontiguousarray(v_all[:, :, h, :]).astype(bf)
        in_maps.append({"qt": qt, "kt": kt, "v": vv, "mask": mask})
    return in_maps


def assemble_output(results):
    """Host: finish denominator (sum over 128 partitions), normalize O^T,
    and undo head sharding."""
    out = np.empty((B, SEQ, N_KV_HEADS, G, D), dtype=np.float32)
    for h in range(N_CORES):
        ot = results[h]["o"].astype(np.float32)       # [B*G, D, SEQ]
        psum = results[h]["ps"].astype(np.float32)    # [B*G, D, SEQ]
        ds = psum.sum(axis=1)                         # [B*G, SEQ]
        oc = ot / ds[:, None, :]                      # normalized O^T
        oc = oc.reshape(B, G, D, SEQ)
        out[:, :, h, :, :] = oc.transpose(0, 3, 1, 2)
    return out.reshape(T, N_QO_HEADS * D)


def kernel(query, key, value, key_cache, value_cache, block_tables,
           new_cache_slots, _trace=False):
    query = np.asarray(query, dtype=np.float32)
    key = np.asarray(key, dtype=np.float32)
    value = np.asarray(value, dtype=np.float32)
    key_cache = np.asarray(key_cache, dtype=np.float32)
    value_cache = np.asarray(value_cache, dtype=np.float32)
    block_tables = np.asarray(block_tables)
    new_cache_slots = np.asarray(new_cache_slots)

    nc = _build_full()
    in_maps = shard_inputs(query, key, value, key_cache, value_cache,
                           block_tables, new_cache_slots)
    res = run_bass_kernel_spmd(
        nc, in_maps, core_ids=list(range(N_CORES)), trace=_trace
    )
    out = assemble_output(res.results)
    if _trace:
        kernel.last_result = res
    return out
